# revision 14
# baseline (speedup 1.0000x reference)
"""Trainium2 Bass kernel for nn_ContinuousCritic (permutation-invariant twin critic).

Strategy: pure data parallel over 8 NeuronCores (batch 32768 -> 4096/core).

Host-side folding: the reference's permutation-stack is affine in the raw
concatenated input x = [obs, ag, g, act] (77 dims + constant-one bias row), so
L1 is fp16 matmuls with folded weights W1eff (K=78), bias as an extra K row.

v3 structure (vs the 175us fp16 baseline: PE 157 / ACT 143 / DVE 133 busy;
the PSUM->SBUF drains are a hard ~130us wall at 1 col/cycle/engine, so the
wins are PE cuts + scheduling):
 - L2 (phi layer 2, K=256) runs in fp8e4 DoubleRow for perms FP8_PERMS
   (2 of 6): one matmul per (half, b-half) instead of two fp16 ones
   (a DR matmul of N=512 is 512 cycles, same as one fp16 matmul).
   Accuracy held by:
     * per-hid-unit scale s[p,t,j] on h1, applied for free via the ACT
       drain's per-partition `scale` AP, folded out of W2;
     * a per-row scale grid-search aligning W2eff to the e4m3 grid;
     * per-output-row scale r[t,m] so W2eff fits e4m3 range, bounded so the
       fp16 acc stays < ~2e4; folded into negb2r / rho1 weights.
   Emulated end-to-end rel err 1.36e-2 (budget 2e-2; fp16 floor 7e-4).
 - rho1 weights are C1-scaled fp16 (R1/r would be fp16-subnormal); undone by
   the rho1 drain's scale=1/C1.
 - rho2 for both twins accumulates into one PSUM tile at partitions 0/32
   (PE col-tiling via out base_partition); the two 1-row drains split
   across ACT and DVE.
 - units run twin-major so each twin's rho1 can start mid-tile; the last
   tile's rho chain overlaps its remaining units (shorter tail).

Layer 2 + fused bias/relu/perm-sum: relu(v + b2) = max(v, -b2) + b2 via one
DVE scalar_tensor_tensor per (unit, half) accumulating into fp16 acc; the
+6*b2 constant is folded into rho1's bias.

Notes from dead ends (measured): DoubleRow + a residual-compensation matmul
is a wash (the comp matmul costs exactly the saved fp16 matmul); K=57
row-paired L1 matmuls via tile_position do NOT overlap on this hardware
(only 32x32 tiles pack; 64-row pairs serialize, K-split concurrent
accumulation is a fatal PSUM write-write collision).
"""

import numpy as np

# ---- problem constants (hardcoded per spec) --------------------------------
B = 32768
N_CORES = 8
BS = B // N_CORES          # 4096 batch per core
TILE = 1024                # batch tile (2 fp32 PSUM banks)
MMN = 512                  # matmul moving size (one fp32 PSUM bank)
NT = BS // TILE            # 4 batch tiles per core
KIN = 78                   # 77 raw features + constant-one row
HID = 256
NPERM = 6
DIM_BODY, DIM_OBJ, NB = 10, 15, 3

FP8_PERMS = (4, 5)         # perm indices (itertools order) using fp8 DoubleRow
PERM_ORDER = (4, 0, 1, 2, 5, 3)  # emission order: fp8 units lead each twin
S0 = 64.0                  # target absmax for quantized h1
W0 = 128.0                 # target absmax for quantized W2eff
ACC0 = 16384.0             # acc (fp16) absmax bound via r
C1 = 64.0                  # rho1 weight scale (kept out of fp16 subnormals)
NSAMP = 2048               # input sample used for scale estimation
NW8 = len(FP8_PERMS)

_PROG = None  # cached program


# ---- host-side math ---------------------------------------------------------

def _perms():
    out = []
    for i in range(NB):
        for j in range(NB):
            if i != j:
                out.append((i, j))
    return out  # [(0,1),(0,2),(1,0),(1,2),(2,0),(2,1)] - matches itertools


def _build_inp_np(obs, ag, g, act):
    """Numpy replica of reference._build_inp."""
    b = obs.shape[0]
    obs_body = obs[:, :DIM_BODY]
    obs_obj = obs[:, DIM_BODY:].reshape(b, NB, DIM_OBJ)
    onehot = np.broadcast_to(np.eye(NB, dtype=obs.dtype), (b, NB, NB))
    feats = np.concatenate([onehot, obs_obj], axis=-1)          # [b,3,18]
    ag_o = ag.reshape(b, NB, NB)                                # OBJ_IDS == reshape
    g_o = g.reshape(b, NB, NB)
    perms = _perms()
    pi = np.array([p[0] for p in perms])
    pj = np.array([p[1] for p in perms])
    body = np.broadcast_to(obs_body[:, None, :], (b, NPERM, DIM_BODY))
    actb = np.broadcast_to(act[:, None, :], (b, NPERM, act.shape[1]))
    inp = np.concatenate([
        ag_o[:, pi], ag_o[:, pj], g_o[:, pi], g_o[:, pj],
        body, feats[:, pi], feats[:, pj], actb], axis=-1)       # [b,6,62]
    return inp


def _affine_maps():
    """inp[b,p] = A[p] @ x_b + c[p] with x = concat(obs, ag, g, act) (77 dims)."""
    X = np.concatenate([np.eye(77, dtype=np.float64),
                        np.zeros((1, 77), dtype=np.float64)], axis=0)  # [78,77]
    obs, ag, g, act = X[:, :55], X[:, 55:64], X[:, 64:73], X[:, 73:77]
    inp = _build_inp_np(obs, ag, g, act)   # [78, 6, 62]
    c = inp[77]                            # [6, 62] constant part
    Ax = inp[:77] - c[None]                # [77, 6, 62]; Ax[k,p,f] = A[p,f,k]
    return Ax, c


def _e4m3(x):
    import ml_dtypes
    ax = np.abs(x).max() if x.size else 0.0
    assert ax < 239.0, f"fp8 overflow risk: absmax {ax}"
    return np.asarray(x, ml_dtypes.float8_e4m3)


_TWIN_KEYS = (
    ("phi_w1a", "phi_b1a", "phi_w2a", "phi_b2a",
     "rho_w1a", "rho_b1a", "rho_w2a", "rho_b2a"),
    ("phi_w1b", "phi_b1b", "phi_w2b", "phi_b2b",
     "rho_w1b", "rho_b1b", "rho_w2b", "rho_b2b"),
)


def _fold_weights(inputs):
    """Host-precompute the packed device weight arrays.

    Returns dict with:
      w1 [78, 3072] f16 : L1 chunks, col block (t*6+p)*256 (W1eff + bias row)
      wb [128, 2052] f16: W2eff16(a|b) kxm-packed (W2*r) | R1eff(a|b)
                          (R1*C1/r) | r2a|r2b
      w8 [128, 2, NW8*512] f8e4: DoubleRow L2 weights; 128-col block
                          ((pi*2+t)*2+h) for perm FP8_PERMS[pi]
      wc [128, 9+NW8*4] f32: negb2r a|b (2 cols each) | rb1e a|b | rho2 bias
                          col (rows 0/32) | h1-scale cols
    """
    Ax, c = _affine_maps()
    f = np.float16
    xt = _build_xt(inputs)                      # [78, B] f16
    xs = xt[:, :NSAMP].astype(np.float32)       # scale-estimation sample
    meta = {}
    E4 = None

    w1 = np.zeros((KIN, 12 * 256), np.float16)
    wb = np.zeros((128, 2052), np.float16)
    w8 = np.zeros((128, 2, NW8 * 512), np.float16)  # cast at end
    wc = np.zeros((128, 9 + NW8 * 4), np.float32)

    def pack_kxm(W):  # [256,256] -> [128, 4*128] with col ((kc*2+h)*128+m)
        return (np.asarray(W, np.float64).reshape(2, 128, 2, 128)
                .transpose(1, 0, 2, 3).reshape(128, 512))

    for t, keys in enumerate(_TWIN_KEYS):
        w1k, b1k, w2k, b2k, r1k, rb1k, r2k, rb2k = keys
        W1 = np.asarray(inputs[w1k], np.float64)
        b1 = np.asarray(inputs[b1k], np.float64)
        W2 = np.asarray(inputs[w2k], np.float64)
        b2 = np.asarray(inputs[b2k], np.float64)
        R1 = np.asarray(inputs[r1k], np.float64)
        rb1 = np.asarray(inputs[rb1k], np.float64)

        W1eff = np.einsum("kpf,fj->pkj", Ax, W1)   # [6,77,256]
        b1eff = c @ W1 + b1[None]                  # [6,256]

        # L1 weight blocks (fp16) + pre-act sample stats
        h1s = [None] * NPERM
        for j, p in enumerate(PERM_ORDER):
            blk = np.concatenate([W1eff[p], b1eff[p][None]], 0)   # [78,256]
            w1[:, (t * 6 + j) * 256:(t * 6 + j + 1) * 256] = blk.astype(f)
            pre = blk.astype(f).astype(np.float32).T @ xs
            h1s[p] = np.maximum(pre, 0.0)
        h1max = np.stack([h.max(1) for h in h1s])                 # [6,256]
        s = S0 / np.maximum(h1max, 1e-3)                          # [6,256]
        wmax = (np.abs(W2)[None] / s[:, :, None]).max((0, 1))     # [256]
        omax = np.zeros(256)
        for p in range(NPERM):
            v = W2.T.astype(np.float32) @ h1s[p]
            omax += np.maximum(v, -b2[:, None]).max(1)
        r = np.minimum(W0 / np.maximum(wmax, 1e-9),
                       ACC0 / np.maximum(omax, 1e-9))             # [256]

        # per-hid-row scale grid-search: align W2eff rows to the e4m3 grid
        import ml_dtypes
        E4 = ml_dtypes.float8_e4m3
        Wr = (W2 * r[None, :]).astype(np.float32)                 # [j, m]
        for p in FP8_PERMS:
            best = np.ones(256)
            berr = np.full(256, np.inf)
            for gck in 2 ** (np.arange(8) / 8.0):
                Wq = (Wr / (s[p][:, None] * gck)).astype(np.float32)
                Qe = np.asarray(Wq, E4).astype(np.float32) - Wq
                e = (Qe ** 2).sum(1) * (s[p] * gck) ** 2
                m2 = e < berr
                berr = np.where(m2, e, berr)
                best = np.where(m2, gck, best)
            s[p] = s[p] * best

        # fp16 L2 weights: W2*r (used by non-fp8 perms)
        wb[:, t * 512:(t + 1) * 512] = pack_kxm(W2 * r[None, :]).astype(f)
        # fp8 DoubleRow blocks
        for pi, p in enumerate(FP8_PERMS):
            W2q = _e4m3((W2 * r[None, :] / s[p][:, None]).astype(np.float32))
            for h in range(2):
                off = ((pi * 2 + t) * 2 + h) * 128
                # [K=128, ko=2, m=128]: value = W2q[ko*128+kp, h*128+m]
                w8[:, :, off:off + 128] = (
                    W2q.astype(np.float32)
                    .reshape(2, 128, 2, 128)[:, :, h]
                    .transpose(1, 0, 2))
                wc[:, 9 + (pi * 2 + t) * 2 + h] = s[p, h * 128:(h + 1) * 128]

        # rho1 weights: R1*C1/r rows, kxm-packed fp16
        wb[:, 1024 + t * 512:1024 + (t + 1) * 512] = \
            pack_kxm(R1 * (C1 / r[:, None])).astype(f)
        # rho2 weights [256] -> [128, 2] col kc
        wb[:, 2048 + 2 * t:2050 + 2 * t] = \
            np.asarray(inputs[r2k], np.float64).reshape(2, 128).T.astype(f)

        # wc: negb2r, rb1e (true scale; drain applies scale=1/C1 before bias)
        wc[:, 2 * t:2 * t + 2] = (-b2 * r).reshape(2, 128).T
        rb1e = rb1 + NPERM * (b2 @ R1)
        wc[:, 4 + 2 * t:6 + 2 * t] = rb1e.reshape(2, 128).T
        wc[32 * t, 8] = np.float32(inputs[rb2k][0])

    meta["w1"] = np.ascontiguousarray(w1)
    meta["wb"] = np.ascontiguousarray(wb)
    meta["w8"] = np.ascontiguousarray(_e4m3(w8.astype(np.float32)))
    meta["wc"] = np.ascontiguousarray(wc)
    return meta


def _build_xt(inputs):
    """xT [78, B]: rows 0..76 = concat(obs, ag, g, act) transposed, row 77 = ones."""
    x = np.concatenate([inputs["obs"], inputs["ag"], inputs["g"], inputs["act"]],
                       axis=1)                     # [B, 77]
    xt = np.zeros((KIN, x.shape[0]), np.float16)
    xt[:77] = np.asarray(x, np.float16).T
    xt[77] = 1.0
    return np.ascontiguousarray(xt)


def numpy_forward(inputs):
    """Folded-math forward in numpy from the PACKED arrays (validates packing)."""
    m = _fold_weights(inputs)
    xt = _build_xt(inputs)
    w1, wb, w8, wc = (m["w1"].astype(np.float32), m["wb"].astype(np.float32),
                      m["w8"].astype(np.float32), m["wc"])
    n = xt.shape[1]
    xf = xt.astype(np.float32)

    def unpack_kxm(cols):  # [128, 512] -> [256, 256]
        return cols.reshape(128, 2, 2, 128).transpose(1, 0, 2, 3).reshape(256, 256)

    qs = []
    for t in range(2):
        negb2r = np.concatenate([wc[:, 2 * t], wc[:, 2 * t + 1]])
        acc = np.zeros((256, n), np.float32)
        W2_16 = unpack_kxm(wb[:, t * 512:(t + 1) * 512])
        for p in range(NPERM):
            j = PERM_ORDER.index(p)
            w1p = w1[:, (t * 6 + j) * 256:(t * 6 + j + 1) * 256]
            ps1 = w1p.T @ xf
            if p in FP8_PERMS:
                pi = FP8_PERMS.index(p)
                sc = np.concatenate([wc[:, 9 + (pi * 2 + t) * 2],
                                     wc[:, 9 + (pi * 2 + t) * 2 + 1]])
                h1 = _e4m3(np.maximum(ps1, 0.0) * sc[:, None]).astype(np.float32)
                ps2 = np.zeros((256, n), np.float32)
                for h in range(2):
                    off = ((pi * 2 + t) * 2 + h) * 128
                    Wq = w8[:, :, off:off + 128]          # [128, 2, 128]
                    for ko in range(2):
                        ps2[h * 128:(h + 1) * 128] += \
                            Wq[:, ko].T @ h1[ko * 128:(ko + 1) * 128]
            else:
                h1 = np.asarray(np.maximum(ps1, 0.0), np.float16) \
                    .astype(np.float32)
                ps2 = W2_16.T @ h1
            acc = (np.maximum(ps2, negb2r[:, None]) + acc).astype(np.float16) \
                .astype(np.float32)
        R1eff = unpack_kxm(wb[:, 1024 + t * 512:1024 + (t + 1) * 512])
        rb1e = np.concatenate([wc[:, 4 + 2 * t], wc[:, 5 + 2 * t]])
        sv = np.maximum(R1eff.T @ acc * np.float32(1.0 / C1) + rb1e[:, None], 0.0)
        sv = np.asarray(sv, np.float16).astype(np.float32)
        R2 = np.concatenate([wb[:, 2048 + 2 * t], wb[:, 2049 + 2 * t]]) \
            .astype(np.float32)
        q = R2[None, :] @ sv + wc[32 * t, 8]
        qs.append(np.ascontiguousarray(q.T, np.float32))        # [B,1]
    return tuple(qs)


# ---- device program ---------------------------------------------------------

def _build_program():
    import concourse.bacc as bacc
    import concourse.mybir as mybir
    import concourse.tile as tile
    from contextlib import ExitStack

    f32 = mybir.dt.float32
    f16 = mybir.dt.float16
    f8 = mybir.dt.float8e4
    RELU = mybir.ActivationFunctionType.Relu
    IDENT = mybir.ActivationFunctionType.Identity
    MAX = mybir.AluOpType.max
    ADD = mybir.AluOpType.add
    MULT = mybir.AluOpType.mult
    DR = mybir.MatmulPerfMode.DoubleRow

    nc = bacc.Bacc("TRN2", target_bir_lowering=False, debug=False)

    xt_d = nc.dram_tensor("xt", [KIN, BS], f16, kind="ExternalInput")
    w1_d = nc.dram_tensor("w1", [KIN, 3072], f16, kind="ExternalInput")
    wb_d = nc.dram_tensor("wb", [128, 2052], f16, kind="ExternalInput")
    w8_d = nc.dram_tensor("w8", [128, 2, NW8 * 512], f8, kind="ExternalInput")
    wc_d = nc.dram_tensor("wc", [128, 9 + NW8 * 4], f32, kind="ExternalInput")
    q_d = nc.dram_tensor("q", [2, BS], f32, kind="ExternalOutput")

    with tile.TileContext(nc) as tc, ExitStack() as ctx:
        wpool = ctx.enter_context(tc.tile_pool(name="wpool", bufs=1))
        xpool = ctx.enter_context(tc.tile_pool(name="xpool", bufs=1))
        h1pool = ctx.enter_context(tc.tile_pool(name="h1pool", bufs=3))
        accpool = ctx.enter_context(tc.tile_pool(name="accpool", bufs=2))
        spool = ctx.enter_context(tc.tile_pool(name="spool", bufs=2))
        qpool = ctx.enter_context(tc.tile_pool(name="qpool", bufs=2))
        pspool = ctx.enter_context(tc.tile_pool(name="pspool", bufs=2, space="PSUM"))

        # preload the ACT spline tables (lazy ACT_TABLE_LOAD would otherwise
        # delay the first real RELU by ~3us) and warm up the PE clock gate.
        warm = wpool.tile([128, 512], f16, tag="warm")
        nc.vector.memset(warm[:], 0.0)
        dummy = wpool.tile([1, 16], f16, tag="dummy")
        nc.scalar.activation(dummy[:], warm[0:1, 0:16],
                             mybir.ActivationFunctionType.Relu)
        for d in range(6):
            psw = pspool.tile([128, TILE], f32, tag=("psL1", "psL2")[d % 2],
                              bufs=2, name=f"warm_{d}")
            nc.tensor.matmul(psw[:, 0:MMN], warm[:, 0:128], warm[:],
                             start=True, stop=True)

        # loads in first-use order; x rides the scalar queue, weights sync.
        x0a = xpool.tile([KIN, MMN], f16, tag="xt0a", name="xt0a")
        nc.scalar.dma_start(x0a[:], xt_d[:, 0:MMN])
        x0b = xpool.tile([KIN, MMN], f16, tag="xt0b", name="xt0b")
        nc.scalar.dma_start(x0b[:], xt_d[:, MMN:TILE])
        w1a = wpool.tile([KIN, 768], f16, tag="w1a")        # t0 first 3 units
        nc.sync.dma_start(w1a[:], w1_d[:, 0:768])
        wcsb = wpool.tile([128, 9 + NW8 * 4], f32, tag="wc")
        nc.sync.dma_start(wcsb[:], wc_d[:])
        xrsb = xpool.tile([KIN, BS - TILE], f16, tag="xtr", name="xtr")
        wb1 = wpool.tile([128, 1024], f16, tag="wb1")       # W2eff16 a|b
        w1b = wpool.tile([KIN, 2304], f16, tag="w1b")       # remaining L1 chunks
        wb2 = wpool.tile([128, 1028], f16, tag="wb2")       # r1a|r1b|r2a|r2b
        w8sb = wpool.tile([128, 2, NW8 * 512], f8, tag="w8")

        def deferred_loads(k):
            if k == 0:
                nc.sync.dma_start(w8sb[:], w8_d[:])
            elif k == 1:
                nc.sync.dma_start(wb1[:], wb_d[:, 0:1024])
            elif k == 2:
                nc.sync.dma_start(w1b[:], w1_d[:, 768:3072])
            elif k == 3:
                nc.scalar.dma_start(xrsb[:], xt_d[:, TILE:BS])
                nc.sync.dma_start(wb2[:], wb_d[:, 1024:2052])

        def negb2_ap(t, h):
            return wcsb[:, 2 * t + h:2 * t + h + 1]

        def w1_ap(p, t, h):
            c = (t * 6 + PERM_ORDER.index(p)) * 256
            if c < 768:
                return w1a[:, c + h * 128:c + (h + 1) * 128]
            return w1b[:, c - 768 + h * 128:c - 768 + (h + 1) * 128]

        def x_ap(i, b):
            if i == 0:
                return (x0a, x0b)[b][:]
            c = (i - 1) * TILE + b * MMN
            return xrsb[:, c:c + MMN]

        # ---- per-unit work (one perm, one twin, one batch tile) ------------
        # Software pipeline: step k emits L1(unit k) then L2(unit k-1).
        def emit_L1(i, p, t, b_outer=False):
            if p in FP8_PERMS:
                h1 = h1pool.tile([128, 2, TILE], f8, tag="h18", bufs=3,
                                 name=f"h18_{p}_{t}")
            else:
                h1 = h1pool.tile([128, 2 * TILE], f16, tag="h1", bufs=4,
                                 name=f"h1_{p}_{t}")
            ps1s = [pspool.tile([128, TILE], f32, tag="psL1", bufs=2,
                                name=f"ps1_{h}")
                    for h in range(2)]
            order = ([(0, 0), (1, 0), (0, 1), (1, 1)] if b_outer else
                     [(0, 0), (0, 1), (1, 0), (1, 1)])
            for h, b in order:
                nc.tensor.matmul(
                    ps1s[h][:, b * MMN:(b + 1) * MMN],
                    w1_ap(p, t, h),
                    x_ap(i, b),
                    start=True, stop=True)
            for h in range(2):
                if p in FP8_PERMS:
                    pi = FP8_PERMS.index(p)
                    sc = wcsb[:, 9 + (pi * 2 + t) * 2 + h:
                              10 + (pi * 2 + t) * 2 + h]
                    if h == 0:
                        nc.scalar.activation(h1[:, h, :], ps1s[h][:], RELU,
                                             scale=sc)
                    else:
                        nc.vector.tensor_scalar(h1[:, h, :], ps1s[h][:],
                                                sc, 0.0, MULT, MAX)
                else:
                    nc.scalar.activation(h1[:, h * TILE:(h + 1) * TILE],
                                         ps1s[h][:], RELU)
            return h1

        def emit_L2(p, t, h1, acc):
            fp8 = p in FP8_PERMS
            pi = FP8_PERMS.index(p) if fp8 else 0
            for h in range(2):
                ps2 = pspool.tile([128, TILE], f32, tag="psL2", bufs=2)
                if fp8:
                    for b in range(2):
                        off = ((pi * 2 + t) * 2 + h) * 128
                        nc.tensor.matmul(
                            ps2[:, b * MMN:(b + 1) * MMN],
                            w8sb[:, :, off:off + 128],
                            h1[:, :, b * MMN:(b + 1) * MMN],
                            start=True, stop=True, perf_mode=DR)
                else:
                    for kc in range(2):
                        for b in range(2):
                            nc.tensor.matmul(
                                ps2[:, b * MMN:(b + 1) * MMN],
                                wb1[:, t * 512 + (kc * 2 + h) * 128:
                                       t * 512 + (kc * 2 + h + 1) * 128],
                                h1[:, kc * TILE + b * MMN:kc * TILE + (b + 1) * MMN],
                                start=(kc == 0), stop=(kc == 1))
                sl = slice(h * TILE, (h + 1) * TILE)
                nc.vector.scalar_tensor_tensor(
                    acc[:, sl], ps2[:], negb2_ap(t, h), acc[:, sl],
                    op0=MAX, op1=ADD)

        # ---- rho actions (queued as soon as their twin's acc completes) ----
        def rho1_half(i, t, h, acc, s):
            ps3 = pspool.tile([128, TILE], f32, tag="psL1", bufs=2)
            for kc in range(2):
                for b in range(2):
                    nc.tensor.matmul(
                        ps3[:, b * MMN:(b + 1) * MMN],
                        wb2[:, t * 512 + (kc * 2 + h) * 128:
                               t * 512 + (kc * 2 + h + 1) * 128],
                        acc[:, kc * TILE + b * MMN:kc * TILE + (b + 1) * MMN],
                        start=(kc == 0), stop=(kc == 1))
            nc.scalar.activation(s[:, h * TILE:(h + 1) * TILE], ps3[:], RELU,
                                 bias=wcsb[:, 4 + 2 * t + h:5 + 2 * t + h],
                                 scale=float(1.0 / C1))

        def rho2_mms(i, ss, psq):
            # both twins into one PSUM tile: twin t at partition 32*t
            for b in range(2):
                for t in range(2):
                    for kc in range(2):
                        nc.tensor.matmul(
                            psq[32 * t:32 * t + 1, b * MMN:(b + 1) * MMN],
                            wb2[:, 1024 + 2 * t + kc:1024 + 2 * t + kc + 1],
                            ss[t][:, kc * TILE + b * MMN:kc * TILE + (b + 1) * MMN],
                            start=(kc == 0), stop=(kc == 1))

        def rho2_drain(i, psq):
            qt = qpool.tile([33, TILE], f32, tag="q", name=f"qt_{i}")
            nc.scalar.activation(qt[0:1, :], psq[0:1, :], IDENT,
                                 bias=wcsb[0:1, 8:9])
            nc.vector.tensor_scalar(qt[32:33, :], psq[32:33, :],
                                    wcsb[32:33, 8:9], None, ADD)
            for t in range(2):
                nc.sync.dma_start(q_d[t:t + 1, i * TILE:(i + 1) * TILE],
                                  qt[32 * t:32 * t + 1, :])

        # ---- main loop: twin-major pipelined emission ----------------------
        units = [(i, t, p) for i in range(NT) for t in range(2)
                 for p in PERM_ORDER]
        LAG = 3
        accs_by_tile = {}
        ss_by_tile = {}
        pend_l2 = []         # (p, t, h1, acc) awaiting L2, depth LAG
        pend_rho = []

        def queue_rho_if_done(idx):
            pi, pt, pp = units[idx]
            if pp != PERM_ORDER[-1]:
                return
            pend_rho.append(
                lambda i_=pi, t_=pt: rho1_half(
                    i_, t_, 0, accs_by_tile[i_][t_], ss_by_tile[i_][t_]))
            pend_rho.append(
                lambda i_=pi, t_=pt: rho1_half(
                    i_, t_, 1, accs_by_tile[i_][t_], ss_by_tile[i_][t_]))
            if pt == 1:
                psq_box = [None]

                def mk_mms(i_=pi, box=psq_box):
                    box[0] = pspool.tile([128, TILE], f32, tag="psL2",
                                         bufs=2, name=f"psq_{i_}")
                    rho2_mms(i_, ss_by_tile[i_], box[0])
                pend_rho.append(mk_mms)
                pend_rho.append(
                    lambda i_=pi, box=psq_box: rho2_drain(i_, box[0]))

        for k, (i, t, p) in enumerate(units):
            if k % 12 == 0:
                accs_by_tile[i] = [
                    accpool.tile([128, 2 * TILE], f16, tag=f"acc{t_}", bufs=2,
                                 name=f"acc{t_}_{i}")
                    for t_ in range(2)]
                for t_ in range(2):
                    nc.gpsimd.memset(accs_by_tile[i][t_][:], 0.0)
                ss_by_tile[i] = [
                    spool.tile([128, 2 * TILE], f16, tag="s", bufs=2,
                               name=f"s{t_}_{i}") for t_ in range(2)]
            h1 = emit_L1(i, p, t, b_outer=(k == 0))
            pend_l2.append((p, t, h1, accs_by_tile[i][t]))
            if len(pend_l2) > LAG:
                emit_L2(*pend_l2.pop(0))
                queue_rho_if_done(k - LAG)
            deferred_loads(k)
            if pend_rho and k % 2 == 0:
                pend_rho.pop(0)()
        k = len(units)
        while pend_l2:
            emit_L2(*pend_l2.pop(0))
            queue_rho_if_done(k - len(pend_l2) - 1)
            if pend_rho:
                pend_rho.pop(0)()
        for a in pend_rho:
            a()

    nc.compile()
    return nc


def _get_program():
    global _PROG
    if _PROG is None:
        _PROG = _build_program()
    return _PROG


# ---- entry points -----------------------------------------------------------

def run(inputs, trace=False):
    from concourse.bass_utils import run_bass_kernel_spmd

    nc = _get_program()
    m = _fold_weights(inputs)
    xt = _build_xt(inputs)

    in_maps = []
    for c in range(N_CORES):
        im = dict(m)
        im["xt"] = np.ascontiguousarray(xt[:, c * BS:(c + 1) * BS])
        in_maps.append(im)

    res = run_bass_kernel_spmd(nc, in_maps, list(range(N_CORES)), trace=trace)
    q = np.concatenate([res.results[c]["q"] for c in range(N_CORES)],
                       axis=1)                      # [2, B]
    qs = tuple(np.ascontiguousarray(q[t].reshape(B, 1), np.float32)
               for t in range(2))
    return qs, res


def kernel(**inputs):
    inputs = {k: np.asarray(v) for k, v in inputs.items()}
    assert inputs["obs"].shape == (B, 55), inputs["obs"].shape
    qs, _ = run(inputs, trace=False)
    return qs


# revision 15
# speedup vs baseline: 1.0465x; 1.0465x over previous
"""Trainium2 Bass kernel for nn_ContinuousCritic (permutation-invariant twin critic).

Strategy: pure data parallel over 8 NeuronCores (batch 32768 -> 4096/core).

Host-side folding: the reference's permutation-stack is affine in the raw
concatenated input x = [obs, ag, g, act] (77 dims + constant-one bias row), so
L1 is fp16 matmuls with folded weights W1eff (K=78), bias as an extra K row.

v3 structure (vs the 175us fp16 baseline: PE 157 / ACT 143 / DVE 133 busy;
the PSUM->SBUF drains are a hard ~130us wall at 1 col/cycle/engine, so the
wins are PE cuts + scheduling):
 - L2 (phi layer 2, K=256) runs in fp8e4 DoubleRow for perms FP8_PERMS
   (2 of 6): one matmul per (half, b-half) instead of two fp16 ones
   (a DR matmul of N=512 is 512 cycles, same as one fp16 matmul).
   Accuracy held by:
     * per-hid-unit scale s[p,t,j] on h1, applied for free via the ACT
       drain's per-partition `scale` AP, folded out of W2;
     * a per-row scale grid-search aligning W2eff to the e4m3 grid;
     * per-output-row scale r[t,m] so W2eff fits e4m3 range, bounded so the
       fp16 acc stays < ~2e4; folded into negb2r / rho1 weights.
   Emulated end-to-end rel err 1.36e-2 (budget 2e-2; fp16 floor 7e-4).
 - rho1 weights are C1-scaled fp16 (R1/r would be fp16-subnormal); undone by
   the rho1 drain's scale=1/C1.
 - rho2 for both twins accumulates into one PSUM tile at partitions 0/32
   (PE col-tiling via out base_partition); the two 1-row drains split
   across ACT and DVE.
 - units run twin-major so each twin's rho1 can start mid-tile; the last
   tile's rho chain overlaps its remaining units (shorter tail).

Layer 2 + fused bias/relu/perm-sum: relu(v + b2) = max(v, -b2) + b2 via one
DVE scalar_tensor_tensor per (unit, half) accumulating into fp16 acc; the
+6*b2 constant is folded into rho1's bias.

Notes from dead ends (measured): DoubleRow + a residual-compensation matmul
is a wash (the comp matmul costs exactly the saved fp16 matmul); K=57
row-paired L1 matmuls via tile_position do NOT overlap on this hardware
(only 32x32 tiles pack; 64-row pairs serialize, K-split concurrent
accumulation is a fatal PSUM write-write collision).
"""

import numpy as np

# ---- problem constants (hardcoded per spec) --------------------------------
B = 32768
N_CORES = 8
BS = B // N_CORES          # 4096 batch per core
TILE = 1024                # batch tile (2 fp32 PSUM banks)
MMN = 512                  # matmul moving size (one fp32 PSUM bank)
NT = BS // TILE            # 4 batch tiles per core
KIN = 78                   # 77 raw features + constant-one row
HID = 256
NPERM = 6
DIM_BODY, DIM_OBJ, NB = 10, 15, 3

FP8_PERMS = (4, 5)         # perm indices (itertools order) using fp8 DoubleRow
PERM_ORDER = (4, 0, 1, 2, 5, 3)  # emission order: fp8 units lead each twin
S0 = 64.0                  # target absmax for quantized h1
W0 = 128.0                 # target absmax for quantized W2eff
ACC0 = 16384.0             # acc (fp16) absmax bound via r
C1 = 64.0                  # rho1 weight scale (kept out of fp16 subnormals)
NSAMP = 2048               # input sample used for scale estimation
NW8 = len(FP8_PERMS)

_PROG = None  # cached program


# ---- host-side math ---------------------------------------------------------

def _perms():
    out = []
    for i in range(NB):
        for j in range(NB):
            if i != j:
                out.append((i, j))
    return out  # [(0,1),(0,2),(1,0),(1,2),(2,0),(2,1)] - matches itertools


def _build_inp_np(obs, ag, g, act):
    """Numpy replica of reference._build_inp."""
    b = obs.shape[0]
    obs_body = obs[:, :DIM_BODY]
    obs_obj = obs[:, DIM_BODY:].reshape(b, NB, DIM_OBJ)
    onehot = np.broadcast_to(np.eye(NB, dtype=obs.dtype), (b, NB, NB))
    feats = np.concatenate([onehot, obs_obj], axis=-1)          # [b,3,18]
    ag_o = ag.reshape(b, NB, NB)                                # OBJ_IDS == reshape
    g_o = g.reshape(b, NB, NB)
    perms = _perms()
    pi = np.array([p[0] for p in perms])
    pj = np.array([p[1] for p in perms])
    body = np.broadcast_to(obs_body[:, None, :], (b, NPERM, DIM_BODY))
    actb = np.broadcast_to(act[:, None, :], (b, NPERM, act.shape[1]))
    inp = np.concatenate([
        ag_o[:, pi], ag_o[:, pj], g_o[:, pi], g_o[:, pj],
        body, feats[:, pi], feats[:, pj], actb], axis=-1)       # [b,6,62]
    return inp


def _affine_maps():
    """inp[b,p] = A[p] @ x_b + c[p] with x = concat(obs, ag, g, act) (77 dims)."""
    X = np.concatenate([np.eye(77, dtype=np.float64),
                        np.zeros((1, 77), dtype=np.float64)], axis=0)  # [78,77]
    obs, ag, g, act = X[:, :55], X[:, 55:64], X[:, 64:73], X[:, 73:77]
    inp = _build_inp_np(obs, ag, g, act)   # [78, 6, 62]
    c = inp[77]                            # [6, 62] constant part
    Ax = inp[:77] - c[None]                # [77, 6, 62]; Ax[k,p,f] = A[p,f,k]
    return Ax, c


def _e4m3(x):
    import ml_dtypes
    ax = np.abs(x).max() if x.size else 0.0
    assert ax < 239.0, f"fp8 overflow risk: absmax {ax}"
    return np.asarray(x, ml_dtypes.float8_e4m3)


_TWIN_KEYS = (
    ("phi_w1a", "phi_b1a", "phi_w2a", "phi_b2a",
     "rho_w1a", "rho_b1a", "rho_w2a", "rho_b2a"),
    ("phi_w1b", "phi_b1b", "phi_w2b", "phi_b2b",
     "rho_w1b", "rho_b1b", "rho_w2b", "rho_b2b"),
)


def _fold_weights(inputs):
    """Host-precompute the packed device weight arrays.

    Returns dict with:
      w1 [78, 3072] f16 : L1 chunks, col block (t*6+p)*256 (W1eff + bias row)
      wb [128, 2052] f16: W2eff16(a|b) kxm-packed (W2*r) | R1eff(a|b)
                          (R1*C1/r) | r2a|r2b
      w8 [128, 2, NW8*512] f8e4: DoubleRow L2 weights; 128-col block
                          ((pi*2+t)*2+h) for perm FP8_PERMS[pi]
      wc [128, 9+NW8*4] f32: negb2r a|b (2 cols each) | rb1e a|b | rho2 bias
                          col (rows 0/32) | h1-scale cols
    """
    Ax, c = _affine_maps()
    f = np.float16
    xt = _build_xt(inputs)                      # [78, B] f16
    xs = xt[:, :NSAMP].astype(np.float32)       # scale-estimation sample
    meta = {}
    E4 = None

    w1 = np.zeros((KIN, 12 * 256), np.float16)
    wb = np.zeros((128, 2052), np.float16)
    w8 = np.zeros((128, 2, NW8 * 512), np.float16)  # cast at end
    wc = np.zeros((128, 9 + NW8 * 4), np.float32)

    def pack_kxm(W):  # [256,256] -> [128, 4*128] with col ((kc*2+h)*128+m)
        return (np.asarray(W, np.float64).reshape(2, 128, 2, 128)
                .transpose(1, 0, 2, 3).reshape(128, 512))

    for t, keys in enumerate(_TWIN_KEYS):
        w1k, b1k, w2k, b2k, r1k, rb1k, r2k, rb2k = keys
        W1 = np.asarray(inputs[w1k], np.float64)
        b1 = np.asarray(inputs[b1k], np.float64)
        W2 = np.asarray(inputs[w2k], np.float64)
        b2 = np.asarray(inputs[b2k], np.float64)
        R1 = np.asarray(inputs[r1k], np.float64)
        rb1 = np.asarray(inputs[rb1k], np.float64)

        W1eff = np.einsum("kpf,fj->pkj", Ax, W1)   # [6,77,256]
        b1eff = c @ W1 + b1[None]                  # [6,256]

        # L1 weight blocks (fp16) + pre-act sample stats
        h1s = [None] * NPERM
        for j, p in enumerate(PERM_ORDER):
            blk = np.concatenate([W1eff[p], b1eff[p][None]], 0)   # [78,256]
            w1[:, (t * 6 + j) * 256:(t * 6 + j + 1) * 256] = blk.astype(f)
            pre = blk.astype(f).astype(np.float32).T @ xs
            h1s[p] = np.maximum(pre, 0.0)
        h1max = np.stack([h.max(1) for h in h1s])                 # [6,256]
        s = S0 / np.maximum(h1max, 1e-3)                          # [6,256]
        wmax = (np.abs(W2)[None] / s[:, :, None]).max((0, 1))     # [256]
        omax = np.zeros(256)
        for p in range(NPERM):
            v = W2.T.astype(np.float32) @ h1s[p]
            omax += np.maximum(v, -b2[:, None]).max(1)
        r = np.minimum(W0 / np.maximum(wmax, 1e-9),
                       ACC0 / np.maximum(omax, 1e-9))             # [256]

        # per-hid-row scale grid-search: align W2eff rows to the e4m3 grid
        import ml_dtypes
        E4 = ml_dtypes.float8_e4m3
        Wr = (W2 * r[None, :]).astype(np.float32)                 # [j, m]
        for p in FP8_PERMS:
            best = np.ones(256)
            berr = np.full(256, np.inf)
            for gck in 2 ** (np.arange(8) / 8.0):
                Wq = (Wr / (s[p][:, None] * gck)).astype(np.float32)
                Qe = np.asarray(Wq, E4).astype(np.float32) - Wq
                e = (Qe ** 2).sum(1) * (s[p] * gck) ** 2
                m2 = e < berr
                berr = np.where(m2, e, berr)
                best = np.where(m2, gck, best)
            s[p] = s[p] * best

        # fp16 L2 weights: W2*r (used by non-fp8 perms)
        wb[:, t * 512:(t + 1) * 512] = pack_kxm(W2 * r[None, :]).astype(f)
        # fp8 DoubleRow blocks
        for pi, p in enumerate(FP8_PERMS):
            W2q = _e4m3((W2 * r[None, :] / s[p][:, None]).astype(np.float32))
            for h in range(2):
                off = ((pi * 2 + t) * 2 + h) * 128
                # [K=128, ko=2, m=128]: value = W2q[ko*128+kp, h*128+m]
                w8[:, :, off:off + 128] = (
                    W2q.astype(np.float32)
                    .reshape(2, 128, 2, 128)[:, :, h]
                    .transpose(1, 0, 2))
                wc[:, 9 + (pi * 2 + t) * 2 + h] = s[p, h * 128:(h + 1) * 128]

        # rho1 weights: R1*C1/r rows, kxm-packed fp16
        wb[:, 1024 + t * 512:1024 + (t + 1) * 512] = \
            pack_kxm(R1 * (C1 / r[:, None])).astype(f)
        # rho2 weights [256] -> [128, 2] col kc
        wb[:, 2048 + 2 * t:2050 + 2 * t] = \
            np.asarray(inputs[r2k], np.float64).reshape(2, 128).T.astype(f)

        # wc: negb2r, rb1e (true scale; drain applies scale=1/C1 before bias)
        wc[:, 2 * t:2 * t + 2] = (-b2 * r).reshape(2, 128).T
        rb1e = rb1 + NPERM * (b2 @ R1)
        wc[:, 4 + 2 * t:6 + 2 * t] = rb1e.reshape(2, 128).T
        wc[32 * t, 8] = np.float32(inputs[rb2k][0])

    meta["w1"] = np.ascontiguousarray(w1)
    meta["wb"] = np.ascontiguousarray(wb)
    meta["w8"] = np.ascontiguousarray(_e4m3(w8.astype(np.float32)))
    meta["wc"] = np.ascontiguousarray(wc)
    return meta


def _build_xt(inputs):
    """xT [78, B]: rows 0..76 = concat(obs, ag, g, act) transposed, row 77 = ones."""
    x = np.concatenate([inputs["obs"], inputs["ag"], inputs["g"], inputs["act"]],
                       axis=1)                     # [B, 77]
    xt = np.zeros((KIN, x.shape[0]), np.float16)
    xt[:77] = np.asarray(x, np.float16).T
    xt[77] = 1.0
    return np.ascontiguousarray(xt)


def numpy_forward(inputs):
    """Folded-math forward in numpy from the PACKED arrays (validates packing)."""
    m = _fold_weights(inputs)
    xt = _build_xt(inputs)
    w1, wb, w8, wc = (m["w1"].astype(np.float32), m["wb"].astype(np.float32),
                      m["w8"].astype(np.float32), m["wc"])
    n = xt.shape[1]
    xf = xt.astype(np.float32)

    def unpack_kxm(cols):  # [128, 512] -> [256, 256]
        return cols.reshape(128, 2, 2, 128).transpose(1, 0, 2, 3).reshape(256, 256)

    qs = []
    for t in range(2):
        negb2r = np.concatenate([wc[:, 2 * t], wc[:, 2 * t + 1]])
        acc = np.zeros((256, n), np.float32)
        W2_16 = unpack_kxm(wb[:, t * 512:(t + 1) * 512])
        for p in range(NPERM):
            j = PERM_ORDER.index(p)
            w1p = w1[:, (t * 6 + j) * 256:(t * 6 + j + 1) * 256]
            ps1 = w1p.T @ xf
            if p in FP8_PERMS:
                pi = FP8_PERMS.index(p)
                sc = np.concatenate([wc[:, 9 + (pi * 2 + t) * 2],
                                     wc[:, 9 + (pi * 2 + t) * 2 + 1]])
                h1 = _e4m3(np.maximum(ps1, 0.0) * sc[:, None]).astype(np.float32)
                ps2 = np.zeros((256, n), np.float32)
                for h in range(2):
                    off = ((pi * 2 + t) * 2 + h) * 128
                    Wq = w8[:, :, off:off + 128]          # [128, 2, 128]
                    for ko in range(2):
                        ps2[h * 128:(h + 1) * 128] += \
                            Wq[:, ko].T @ h1[ko * 128:(ko + 1) * 128]
            else:
                h1 = np.asarray(np.maximum(ps1, 0.0), np.float16) \
                    .astype(np.float32)
                ps2 = W2_16.T @ h1
            acc = (np.maximum(ps2, negb2r[:, None]) + acc).astype(np.float16) \
                .astype(np.float32)
        R1eff = unpack_kxm(wb[:, 1024 + t * 512:1024 + (t + 1) * 512])
        rb1e = np.concatenate([wc[:, 4 + 2 * t], wc[:, 5 + 2 * t]])
        sv = np.maximum(R1eff.T @ acc * np.float32(1.0 / C1) + rb1e[:, None], 0.0)
        sv = np.asarray(sv, np.float16).astype(np.float32)
        R2 = np.concatenate([wb[:, 2048 + 2 * t], wb[:, 2049 + 2 * t]]) \
            .astype(np.float32)
        q = R2[None, :] @ sv + wc[32 * t, 8]
        qs.append(np.ascontiguousarray(q.T, np.float32))        # [B,1]
    return tuple(qs)


# ---- device program ---------------------------------------------------------

def _build_program():
    import concourse.bacc as bacc
    import concourse.mybir as mybir
    import concourse.tile as tile
    from contextlib import ExitStack

    f32 = mybir.dt.float32
    f16 = mybir.dt.float16
    f8 = mybir.dt.float8e4
    RELU = mybir.ActivationFunctionType.Relu
    IDENT = mybir.ActivationFunctionType.Identity
    MAX = mybir.AluOpType.max
    ADD = mybir.AluOpType.add
    MULT = mybir.AluOpType.mult
    DR = mybir.MatmulPerfMode.DoubleRow

    nc = bacc.Bacc("TRN2", target_bir_lowering=False, debug=False)

    xt_d = nc.dram_tensor("xt", [KIN, BS], f16, kind="ExternalInput")
    w1_d = nc.dram_tensor("w1", [KIN, 3072], f16, kind="ExternalInput")
    wb_d = nc.dram_tensor("wb", [128, 2052], f16, kind="ExternalInput")
    w8_d = nc.dram_tensor("w8", [128, 2, NW8 * 512], f8, kind="ExternalInput")
    wc_d = nc.dram_tensor("wc", [128, 9 + NW8 * 4], f32, kind="ExternalInput")
    q_d = nc.dram_tensor("q", [2, BS], f32, kind="ExternalOutput")

    with tile.TileContext(nc) as tc, ExitStack() as ctx:
        wpool = ctx.enter_context(tc.tile_pool(name="wpool", bufs=1))
        xpool = ctx.enter_context(tc.tile_pool(name="xpool", bufs=1))
        h1pool = ctx.enter_context(tc.tile_pool(name="h1pool", bufs=3))
        accpool = ctx.enter_context(tc.tile_pool(name="accpool", bufs=2))
        spool = ctx.enter_context(tc.tile_pool(name="spool", bufs=2))
        qpool = ctx.enter_context(tc.tile_pool(name="qpool", bufs=2))
        pspool = ctx.enter_context(tc.tile_pool(name="pspool", bufs=2, space="PSUM"))

        # preload the ACT spline tables (lazy ACT_TABLE_LOAD would otherwise
        # delay the first real RELU by ~3us) and warm up the PE clock gate.
        warm = wpool.tile([128, 512], f16, tag="warm")
        nc.vector.memset(warm[:], 0.0)
        dummy = wpool.tile([1, 16], f16, tag="dummy")
        nc.scalar.activation(dummy[:], warm[0:1, 0:16],
                             mybir.ActivationFunctionType.Relu)
        for d in range(6):
            psw = pspool.tile([128, TILE], f32, tag=("psL1", "psL2")[d % 2],
                              bufs=2, name=f"warm_{d}")
            nc.tensor.matmul(psw[:, 0:MMN], warm[:, 0:128], warm[:],
                             start=True, stop=True)

        # loads in first-use order; x rides the scalar queue, weights sync.
        x0a = xpool.tile([KIN, MMN], f16, tag="xt0a", name="xt0a")
        nc.scalar.dma_start(x0a[:], xt_d[:, 0:MMN])
        x0b = xpool.tile([KIN, MMN], f16, tag="xt0b", name="xt0b")
        nc.scalar.dma_start(x0b[:], xt_d[:, MMN:TILE])
        w1a = wpool.tile([KIN, 768], f16, tag="w1a")        # t0 first 3 units
        nc.sync.dma_start(w1a[:], w1_d[:, 0:768])
        wcsb = wpool.tile([128, 9 + NW8 * 4], f32, tag="wc")
        nc.sync.dma_start(wcsb[:], wc_d[:])
        xrsb = xpool.tile([KIN, BS - TILE], f16, tag="xtr", name="xtr")
        wb1 = wpool.tile([128, 1024], f16, tag="wb1")       # W2eff16 a|b
        w1b = wpool.tile([KIN, 2304], f16, tag="w1b")       # remaining L1 chunks
        wb2 = wpool.tile([128, 1028], f16, tag="wb2")       # r1a|r1b|r2a|r2b
        w8sb = wpool.tile([128, 2, NW8 * 512], f8, tag="w8")

        def deferred_loads(k):
            if k == 0:
                nc.sync.dma_start(w8sb[:], w8_d[:])
            elif k == 1:
                nc.sync.dma_start(wb1[:], wb_d[:, 0:1024])
            elif k == 2:
                nc.sync.dma_start(w1b[:], w1_d[:, 768:3072])
            elif k == 3:
                nc.scalar.dma_start(xrsb[:], xt_d[:, TILE:BS])
                nc.sync.dma_start(wb2[:], wb_d[:, 1024:2052])

        def negb2_ap(t, h):
            return wcsb[:, 2 * t + h:2 * t + h + 1]

        def w1_ap(p, t, h):
            c = (t * 6 + PERM_ORDER.index(p)) * 256
            if c < 768:
                return w1a[:, c + h * 128:c + (h + 1) * 128]
            return w1b[:, c - 768 + h * 128:c - 768 + (h + 1) * 128]

        def x_ap(i, b):
            if i == 0:
                return (x0a, x0b)[b][:]
            c = (i - 1) * TILE + b * MMN
            return xrsb[:, c:c + MMN]

        # ---- per-unit work (one perm, one twin, one batch tile) ------------
        # Software pipeline: step k emits L1(unit k) then L2(unit k-1).
        def emit_L1(i, p, t, b_outer=False):
            if p in FP8_PERMS:
                h1 = h1pool.tile([128, 2, TILE], f8, tag="h18", bufs=3,
                                 name=f"h18_{p}_{t}")
            else:
                h1 = h1pool.tile([128, 2 * TILE], f16, tag="h1", bufs=4,
                                 name=f"h1_{p}_{t}")
            ps1s = [pspool.tile([128, TILE], f32, tag="psL1", bufs=2,
                                name=f"ps1_{h}")
                    for h in range(2)]
            order = ([(0, 0), (1, 0), (0, 1), (1, 1)] if b_outer else
                     [(0, 0), (0, 1), (1, 0), (1, 1)])
            for h, b in order:
                nc.tensor.matmul(
                    ps1s[h][:, b * MMN:(b + 1) * MMN],
                    w1_ap(p, t, h),
                    x_ap(i, b),
                    start=True, stop=True)
            for h in range(2):
                if p in FP8_PERMS:
                    pi = FP8_PERMS.index(p)
                    sc = wcsb[:, 9 + (pi * 2 + t) * 2 + h:
                              10 + (pi * 2 + t) * 2 + h]
                    nc.scalar.activation(h1[:, h, :], ps1s[h][:], RELU,
                                         scale=sc)
                else:
                    nc.scalar.activation(h1[:, h * TILE:(h + 1) * TILE],
                                         ps1s[h][:], RELU)
            return h1

        def emit_L2(p, t, h1, acc):
            fp8 = p in FP8_PERMS
            pi = FP8_PERMS.index(p) if fp8 else 0
            for h in range(2):
                ps2 = pspool.tile([128, TILE], f32, tag="psL2", bufs=2)
                if fp8:
                    for b in range(2):
                        off = ((pi * 2 + t) * 2 + h) * 128
                        nc.tensor.matmul(
                            ps2[:, b * MMN:(b + 1) * MMN],
                            w8sb[:, :, off:off + 128],
                            h1[:, :, b * MMN:(b + 1) * MMN],
                            start=True, stop=True, perf_mode=DR)
                else:
                    for kc in range(2):
                        for b in range(2):
                            nc.tensor.matmul(
                                ps2[:, b * MMN:(b + 1) * MMN],
                                wb1[:, t * 512 + (kc * 2 + h) * 128:
                                       t * 512 + (kc * 2 + h + 1) * 128],
                                h1[:, kc * TILE + b * MMN:kc * TILE + (b + 1) * MMN],
                                start=(kc == 0), stop=(kc == 1))
                sl = slice(h * TILE, (h + 1) * TILE)
                if p == PERM_ORDER[0]:
                    nc.vector.tensor_scalar(
                        acc[:, sl], ps2[:], negb2_ap(t, h), None, MAX)
                else:
                    nc.vector.scalar_tensor_tensor(
                        acc[:, sl], ps2[:], negb2_ap(t, h), acc[:, sl],
                        op0=MAX, op1=ADD)

        # ---- rho actions (queued as soon as their twin's acc completes) ----
        def rho1_half(i, t, h, acc, s):
            ps3 = pspool.tile([128, TILE], f32, tag="psL1", bufs=2)
            for kc in range(2):
                for b in range(2):
                    nc.tensor.matmul(
                        ps3[:, b * MMN:(b + 1) * MMN],
                        wb2[:, t * 512 + (kc * 2 + h) * 128:
                               t * 512 + (kc * 2 + h + 1) * 128],
                        acc[:, kc * TILE + b * MMN:kc * TILE + (b + 1) * MMN],
                        start=(kc == 0), stop=(kc == 1))
            nc.scalar.activation(s[:, h * TILE:(h + 1) * TILE], ps3[:], RELU,
                                 bias=wcsb[:, 4 + 2 * t + h:5 + 2 * t + h],
                                 scale=float(1.0 / C1))

        def rho2_mms(i, ss, psq):
            # both twins into one PSUM tile: twin t at partition 32*t
            for b in range(2):
                for t in range(2):
                    for kc in range(2):
                        nc.tensor.matmul(
                            psq[32 * t:32 * t + 1, b * MMN:(b + 1) * MMN],
                            wb2[:, 1024 + 2 * t + kc:1024 + 2 * t + kc + 1],
                            ss[t][:, kc * TILE + b * MMN:kc * TILE + (b + 1) * MMN],
                            start=(kc == 0), stop=(kc == 1))

        def rho2_drain(i, psq):
            qt = qpool.tile([33, TILE], f32, tag="q", name=f"qt_{i}")
            nc.scalar.activation(qt[0:1, :], psq[0:1, :], IDENT,
                                 bias=wcsb[0:1, 8:9])
            nc.vector.tensor_scalar(qt[32:33, :], psq[32:33, :],
                                    wcsb[32:33, 8:9], None, ADD)
            for t in range(2):
                nc.sync.dma_start(q_d[t:t + 1, i * TILE:(i + 1) * TILE],
                                  qt[32 * t:32 * t + 1, :])

        # ---- main loop: twin-major pipelined emission ----------------------
        units = [(i, t, p) for i in range(NT) for t in range(2)
                 for p in PERM_ORDER]
        LAG = 3
        accs_by_tile = {}
        ss_by_tile = {}
        pend_l2 = []         # (p, t, h1, acc) awaiting L2, depth LAG
        pend_rho = []

        def queue_rho_if_done(idx):
            pi, pt, pp = units[idx]
            if pp != PERM_ORDER[-1]:
                return
            pend_rho.append(
                lambda i_=pi, t_=pt: rho1_half(
                    i_, t_, 0, accs_by_tile[i_][t_], ss_by_tile[i_][t_]))
            pend_rho.append(
                lambda i_=pi, t_=pt: rho1_half(
                    i_, t_, 1, accs_by_tile[i_][t_], ss_by_tile[i_][t_]))
            if pt == 1:
                psq_box = [None]

                def mk_mms(i_=pi, box=psq_box):
                    box[0] = pspool.tile([128, TILE], f32, tag="psL2",
                                         bufs=2, name=f"psq_{i_}")
                    rho2_mms(i_, ss_by_tile[i_], box[0])
                pend_rho.append(mk_mms)
                pend_rho.append(
                    lambda i_=pi, box=psq_box: rho2_drain(i_, box[0]))

        for k, (i, t, p) in enumerate(units):
            if k % 12 == 0:
                accs_by_tile[i] = [
                    accpool.tile([128, 2 * TILE], f16, tag=f"acc{t_}", bufs=2,
                                 name=f"acc{t_}_{i}")
                    for t_ in range(2)]
                ss_by_tile[i] = [
                    spool.tile([128, 2 * TILE], f16, tag="s", bufs=2,
                               name=f"s{t_}_{i}") for t_ in range(2)]
            h1 = emit_L1(i, p, t, b_outer=(k == 0))
            pend_l2.append((p, t, h1, accs_by_tile[i][t]))
            if len(pend_l2) > LAG:
                emit_L2(*pend_l2.pop(0))
                queue_rho_if_done(k - LAG)
            deferred_loads(k)
            if pend_rho and k % 2 == 0:
                pend_rho.pop(0)()
        k = len(units)
        while pend_l2:
            emit_L2(*pend_l2.pop(0))
            queue_rho_if_done(k - len(pend_l2) - 1)
            if pend_rho:
                pend_rho.pop(0)()
        for a in pend_rho:
            a()

    nc.compile()
    return nc


def _get_program():
    global _PROG
    if _PROG is None:
        _PROG = _build_program()
    return _PROG


# ---- entry points -----------------------------------------------------------

def run(inputs, trace=False):
    from concourse.bass_utils import run_bass_kernel_spmd

    nc = _get_program()
    m = _fold_weights(inputs)
    xt = _build_xt(inputs)

    in_maps = []
    for c in range(N_CORES):
        im = dict(m)
        im["xt"] = np.ascontiguousarray(xt[:, c * BS:(c + 1) * BS])
        in_maps.append(im)

    res = run_bass_kernel_spmd(nc, in_maps, list(range(N_CORES)), trace=trace)
    q = np.concatenate([res.results[c]["q"] for c in range(N_CORES)],
                       axis=1)                      # [2, B]
    qs = tuple(np.ascontiguousarray(q[t].reshape(B, 1), np.float32)
               for t in range(2))
    return qs, res


def kernel(**inputs):
    inputs = {k: np.asarray(v) for k, v in inputs.items()}
    assert inputs["obs"].shape == (B, 55), inputs["obs"].shape
    qs, _ = run(inputs, trace=False)
    return qs
